# revision 39
# baseline (speedup 1.0000x reference)
"""Trainium2 Bass kernel for nn_MultiHeadAttention (B=2, S=4096, F=512, H=8, causal).

Sharding: 8 cores = 2 (batch) x 4 (head pairs). Each core computes the
projections for its 2 heads, causal flash-style attention with logits in
[Sk, Sq] (transposed) layout, and its normalized partial output
projection. The host pre-transposes q/k/v per batch (bf16), slices
weights per head pair (bf16), sums the 4 partial outputs per batch
(replaces the all-reduce) and adds bv @ wo + bo (exact because softmax
rows sum to 1).

Design (validated on HW against several alternatives):
- All matmul inputs in bf16 (half DMA traffic, no f32r rounding copies,
  FWL weight loads). PSUM stays f32; the final output is f32.
- Per-head QK^T matmul pairs occupy disjoint PE row halves (tile_position
  (0,0)/(64,0) auto-derived) so HW runs them concurrently; same for the
  output-projection pair (attn heads packed into rows 0:64 / 64:128).
- The single ones column shared by both heads' overlapping 128-wide PV
  stationary windows accumulates the softmax denominators into psum
  rows 64 (head A) and 0 (head B); they ride the attention evacuation
  copies and are pulled into per-partition layout by 1-contraction
  matmuls borrowing col 0 of the output-projection banks.
- exp() runs entirely on ACT (table exp, scale+bias fused); mask
  multiplies alternate DVE (bf16 2x) / GPSIMD. (A custom 2-instruction
  DVE exp — cubic core + 6 squarings, ~1.5e-3 — is implemented and
  env-togglable, but measured slower on HW: it ties up an lt PSUM
  buffer ~2x longer per item and stalls the QK->exp->PV chain.)
- Software pipeline: the next block's input DMAs and projections are
  emitted ahead of the current block's attention items; the output
  projection runs one block behind as a tail burst; pools span the rep
  loop so the pipeline flows across reps.

The causal structure is not hardcoded: the mask input is classified on
the host into full / partial / skipped [128 x 512] tiles and the device
program is built (and cached) from that schedule, so any 0/1-style
additive mask (including all-zeros) produces a correct program.
"""

import numpy as np
import ml_dtypes
from contextlib import ExitStack

import concourse.bass as bass
import concourse.tile as tile
from concourse import bacc, mybir
from concourse import bass2jax
from concourse import dve_ops
from concourse.dve_ops import DveOp
from concourse.dve_spec import Spec, lower, Src0, Src1, C0, C1, C2, One, sq
import concourse.dve_spec as dspec
from concourse.dve_uop import DveOpSpec

F32 = mybir.dt.float32
BF16 = mybir.dt.bfloat16
AF = mybir.ActivationFunctionType
ALU = mybir.AluOpType

B = 2
S = 4096
NF = 512
NH = 8
D = 64
N_CORES = 8
SQ = 512          # query block width
SK = 128          # key tile height
N_QB = S // SQ    # 8
N_SKT = S // SK   # 32
SCALE = 1.0 / np.sqrt(np.float32(D))  # 0.125
EXP_BIAS = -4.0   # constant shift inside exp; cancels in normalization

# exp(SCALE*x + EXP_BIAS) = (q*((q+G)^2+1))^64 with q = EC0*x + EC1.
# Cubic-core constants fit over u = (SCALE*x + EXP_BIAS)/64 in
# [-0.235, 0.110] (raw |logit| <= ~75); max rel err ~1.5e-3 after ^64.
EC0 = 0.0010872830171138048
EC1 = 0.842393159866333
EG = -0.5030438899993896

# exp routing: full tiles go to the DVE custom op every Nth item
# (rest on ACT); masked tiles: ACT exp + DVE bf16 mask-multiply.
import os
# Measured on HW: keeping the full-tile exp stream entirely on ACT
# pipelines best (the 2-pass DVE exp ties up an lt PSUM buffer ~2x longer
# per item and stalls the QK->exp->PV chain more than it relieves ACT).
DVE_FULL_EVERY = int(os.environ.get("K_DVE_FULL_EVERY", "1000000"))
# mask multiplies alternate between DVE (bf16 2x) and GPSIMD
MASK_ON_GPSIMD_EVERY = int(os.environ.get("K_MASK_GPSIMD_EVERY", "2"))
# masked tiles: exp+mask fused on DVE (1) vs ACT exp + mask mult (0)
MASKED_ON_DVE = int(os.environ.get("K_MASKED_DVE", "0"))

_CACHE: dict = {}
_OPS: dict = {}


def _register_dve_ops():
    """Register the custom DVE exp ops (idempotent)."""
    if _OPS:
        return _OPS
    _q = Src0 * C0 + C1
    core_body = (sq(_q + C2) + One) * _q

    def _np_core(x, c0, c1, g):
        x = np.asarray(x, np.float32)
        qq = np.float32(c0) * x + np.float32(c1)
        return ((qq + np.float32(g)) ** 2 + np.float32(1.0)) * qq

    specs = [
        ("EXP_CORE_ANT", Spec(
            body=core_body,
            reference=lambda in0, in1, s0, s1, imm2: _np_core(in0, s0, s1, imm2),
        )),
        ("EXP_CORE_MASK_ANT", Spec(
            body=core_body * Src1,
            reference=lambda in0, in1, s0, s1, imm2: _np_core(in0, s0, s1, imm2)
            * np.asarray(in1, np.float32),
        )),
        ("EXP_SQ6_ANT", Spec(
            body=sq(sq(sq(sq(sq(sq(Src0)))))),
            reference=lambda in0, in1, s0, s1, imm2: (
                np.asarray(in0, np.float32) ** 64),
        )),
        # out = in0*s0 + in1*s1 with per-partition scalars: the fused
        # two-head output normalize+combine.
        ("OUT_COMBINE_ANT", Spec(
            body=Src0 * C0 + Src1 * C1,
            reference=lambda in0, in1, s0, s1, imm2: (
                np.asarray(in0, np.float32) * s0
                + np.asarray(in1, np.float32) * s1),
        )),
    ]
    for name, spec in specs:
        if name not in dve_ops._SUB_OPCODE_FOR_NAME:
            row = max(dve_ops._SUB_OPCODE_FOR_NAME.values()) + 1
            assert row < 0x20
            op = DveOp(name, spec, subdim=False, uops_sha={})
            for ver in ("v3", "v4"):
                s = DveOpSpec(name=name, opcode=row,
                              uops=lower(spec, ver=ver),
                              rd1_en=dspec._has_src1(spec))
                op.uops_sha[ver] = s.sha(ver)
            dve_ops.OPS.append(op)
            dve_ops.CUSTOM_DVE_SPECS[name] = spec
            dve_ops._SUB_OPCODE_FOR_NAME[name] = row
        _OPS[name] = next(o for o in dve_ops.OPS if o.name == name)
    return _OPS


def _classify_mask(mask: np.ndarray):
    """mask: [S, S] additive-style (nonzero => disallowed).

    Returns (schedule, patterns):
      schedule[qb] = list of (sk, qlo, pat_idx_or_None)
      patterns: np.ndarray [n_pat, 128, 512] of multiplicative 0/1 masks.
    """
    m = mask != 0
    schedule = []
    patterns = []
    pat_index: dict = {}
    for qb in range(N_QB):
        items = []
        for sk in range(N_SKT):
            sub = m[qb * SQ:(qb + 1) * SQ, sk * SK:(sk + 1) * SK].T
            if sub.all():
                continue
            if not sub.any():
                items.append((sk, 0, None))
                continue
            col_full_masked = sub.all(axis=0)
            qlo = int(np.argmax(~col_full_masked))
            qlo = (qlo // 128) * 128
            pat = (~sub).astype(np.float32)  # 1 = allowed
            key = pat.tobytes()
            if key not in pat_index:
                pat_index[key] = len(patterns)
                patterns.append(pat)
            items.append((sk, qlo, pat_index[key]))
        schedule.append(tuple(items))
    pats = np.stack(patterns) if patterns else np.ones((1, SK, SQ), np.float32)
    return tuple(schedule), pats


def _build_program(schedule, n_pat, reps=1):
    ops = _register_dve_ops()
    core_op = ops["EXP_CORE_ANT"]
    mask_op = ops["EXP_CORE_MASK_ANT"]
    sq6_op = ops["EXP_SQ6_ANT"]

    nc = bacc.Bacc("TRN2", target_bir_lowering=False, debug=False,
                   num_devices=N_CORES)

    qT = nc.dram_tensor("qT", [NF, S], BF16, kind="ExternalInput").ap()
    kT = nc.dram_tensor("kT", [NF, S], BF16, kind="ExternalInput").ap()
    vT = nc.dram_tensor("vT", [NF, S], BF16, kind="ExternalInput").ap()
    wq_d = nc.dram_tensor("wq", [NF, 128], BF16, kind="ExternalInput").ap()
    wk_d = nc.dram_tensor("wk", [NF, 128], BF16, kind="ExternalInput").ap()
    wv_d = nc.dram_tensor("wv", [NF, 128], BF16, kind="ExternalInput").ap()
    wo_d = nc.dram_tensor("wo", [128, NF], BF16, kind="ExternalInput").ap()
    bq_d = nc.dram_tensor("bq", [128, 1], F32, kind="ExternalInput").ap()
    bk_d = nc.dram_tensor("bk", [128, 1], F32, kind="ExternalInput").ap()
    msk_d = nc.dram_tensor("msk", [SK, n_pat * 2 * SQ], BF16,
                           kind="ExternalInput").ap()
    o_d = nc.dram_tensor("o", [S, NF], F32, kind="ExternalOutput").ap()

    with tile.TileContext(nc) as tc, ExitStack() as octx:
        per = octx.enter_context(tc.tile_pool(name="persist", bufs=1))

        QhT = per.tile([128, S], BF16, tag="qh")      # [head dims (A|B), S]
        KhT = per.tile([128, S], BF16, tag="kh")
        # PV stationaries, overlapping 128-wide windows per sk tile:
        #   cols 0:64 = A dims, col 64 = ones, 65:128 = 0, 128:192 = B dims
        #   A window = cols 0:128  (den -> psum row 64, attn rows 0:64)
        #   B window = cols 64:192 (den -> psum row 0, attn rows 64:128)
        # The single ones column serves both heads.
        Vaug = per.tile([128, N_SKT, 256], BF16, tag="vaug")
        # attnA: rows 0:64 attn, 64 = denA; attnB: row 0 = denB,
        # rows 64:128 attn (matches psum layout; lane-aligned copies).
        attnA = per.tile([128, S], BF16, tag="attnA")
        attnB = per.tile([128, S], BF16, tag="attnB")
        wq_sb = per.tile([128, 4, 128], BF16, tag="wq")
        wk_sb = per.tile([128, 4, 128], BF16, tag="wk")
        wv_sb = per.tile([128, 4, 128], BF16, tag="wv")
        wo_sb = per.tile([128, NF], BF16, tag="wo")
        bq_sb = per.tile([128, 1], F32, tag="bq")
        bk_sb = per.tile([128, 1], F32, tag="bk")
        msk_sb = per.tile([SK, n_pat, 2, SQ], BF16, tag="msk")
        ebias = per.tile([128, 1], F32, tag="ebias")
        ones_sb = per.tile([128, 1], BF16, tag="ones")

        nc.vector.memset(ebias, EXP_BIAS)
        nc.vector.memset(ones_sb, 1.0)
        nc.vector.memset(Vaug, 0.0)
        nc.vector.memset(Vaug[:, :, 64:65], 1.0)
        nc.sync.dma_start(wq_sb, wq_d.rearrange("(c p) m -> p c m", p=128))
        nc.sync.dma_start(wk_sb, wk_d.rearrange("(c p) m -> p c m", p=128))
        nc.sync.dma_start(wv_sb, wv_d.rearrange("(c p) m -> p c m", p=128))
        nc.sync.dma_start(wo_sb, wo_d)
        nc.sync.dma_start(bq_sb, bq_d)
        nc.sync.dma_start(bk_sb, bk_d)
        nc.sync.dma_start(
            msk_sb, msk_d.rearrange("k (p two q) -> k p two q", two=2, q=SQ))

        # PSUM banks: shared proj/oproj pool 2, lt 2x2=4, pv 2 -> 8.
        # Pools span the rep loop so the pipeline flows across reps.
        with tc.tile_pool(name="xs", bufs=3) as xs, \
             tc.tile_pool(name="ps2", bufs=2, space="PSUM") as ps2, \
             tc.tile_pool(name="pp", bufs=4) as pp, \
             tc.tile_pool(name="tp", bufs=2) as tp, \
             tc.tile_pool(name="ltp", bufs=2, space="PSUM") as ltp, \
             tc.tile_pool(name="pvp", bufs=2, space="PSUM") as pvp:
            dve_ctr = [0]
            msk_ctr = [0]

            def proj_dma(qb):
                """Issue the three input-block loads (early, so they
                prefetch ahead of the compute that consumes them)."""
                qsl = slice(qb * SQ, (qb + 1) * SQ)
                xk = xs.tile([128, 4, SQ], BF16, tag="x", bufs=4)
                nc.sync.dma_start(
                    xk, kT.rearrange("(c p) m -> p c m", p=128)[:, :, qsl])
                xq = xs.tile([128, 4, SQ], BF16, tag="x", bufs=4)
                nc.sync.dma_start(
                    xq, qT.rearrange("(c p) m -> p c m", p=128)[:, :, qsl])
                vb = xs.tile([128, 4, SQ], BF16, tag="x", bufs=4)
                nc.sync.dma_start(
                    vb, vT.rearrange("(c p) m -> p c m", p=128)[:, :, qsl])
                return xk, xq, vb

            def proj_mm(qb, tiles):
                qsl = slice(qb * SQ, (qb + 1) * SQ)
                xk, xq, vb = tiles
                for dst, xb, w_s, b_s in ((KhT, xk, wk_sb, bk_sb),
                                          (QhT, xq, wq_sb, bq_sb)):
                    pt = ps2.tile([128, SQ], F32, tag="ps")
                    for f in range(4):
                        nc.tensor.matmul(pt, w_s[:, f, :], xb[:, f, :],
                                         start=(f == 0), stop=(f == 3))
                    nc.vector.tensor_scalar_add(dst[:, qsl], pt, b_s)
                # V: x-stationary so psum comes out [s, d]; one strided
                # copy per 128-s chunk drops A dims into cols 0:64 and
                # B dims into cols 128:192.
                for j in range(4):
                    st = 4 * qb + j
                    pv_ = ps2.tile([128, 128], F32, tag="ps")
                    for f in range(4):
                        nc.tensor.matmul(pv_, vb[:, f, j * 128:(j + 1) * 128],
                                         wv_sb[:, f, :],
                                         start=(f == 0), stop=(f == 3))
                    nc.vector.tensor_copy(
                        Vaug[:, st, 0:256].rearrange(
                            "p (a b) -> p a b", a=2)[:, :, 0:64],
                        pv_.rearrange("p (a b) -> p a b", a=2))

            def emit_item(qb, idx, n_items, pvA, pvB):
                q0 = qb * SQ
                sk, qlo, pat = schedule[qb][idx]
                ksl = slice(sk * SK, (sk + 1) * SK)
                qs = slice(q0 + qlo, q0 + SQ)
                lt = ltp.tile([128, 1024], F32, tag="lt")
                pAB = pp.tile([128, 1024], BF16, tag="pAB")
                nc.tensor.matmul(lt[:, qlo:SQ], KhT[0:64, ksl],
                                 QhT[0:64, qs], start=True, stop=True)
                nc.tensor.matmul(lt[:, SQ + qlo:2 * SQ], KhT[64:128, ksl],
                                 QhT[64:128, qs], start=True, stop=True)
                if pat is None:
                    # full tile: route exp to ACT or DVE
                    dve_ctr[0] += 1
                    if dve_ctr[0] % DVE_FULL_EVERY == 0:
                        tmp = tp.tile([128, 1024], F32, tag="tmp")
                        nc.vector._custom_dve(
                            core_op, out=tmp, in0=lt,
                            s0=EC0, s1=EC1, imm2=EG)
                        nc.vector._custom_dve(sq6_op, out=pAB, in0=tmp)
                    else:
                        nc.scalar.activation(pAB, lt, AF.Exp,
                                             bias=ebias, scale=float(SCALE))
                elif MASKED_ON_DVE:
                    tmp = tp.tile([128, 1024], F32, tag="tmp")
                    t3 = tmp.rearrange("p (two q) -> p two q",
                                       q=SQ)[:, :, qlo:SQ]
                    oap = pAB.rearrange("p (two q) -> p two q",
                                        q=SQ)[:, :, qlo:SQ]
                    for h in range(2):
                        nc.vector._custom_dve(
                            mask_op, out=t3[:, h, :],
                            in0=lt[:, h * SQ + qlo:(h + 1) * SQ],
                            in1=msk_sb[:, pat, h, qlo:SQ],
                            s0=EC0, s1=EC1, imm2=EG)
                    nc.vector._custom_dve(sq6_op, out=oap, in0=t3)
                else:
                    oap = pAB.rearrange("p (two q) -> p two q",
                                        q=SQ)[:, :, qlo:SQ]
                    iap = lt.rearrange("p (two q) -> p two q",
                                       q=SQ)[:, :, qlo:SQ]
                    msl = msk_sb[:, pat, :, qlo:SQ]
                    nc.scalar.activation(oap, iap, AF.Exp,
                                         bias=ebias, scale=float(SCALE))
                    msk_ctr[0] += 1
                    if msk_ctr[0] % MASK_ON_GPSIMD_EVERY == 0:
                        nc.gpsimd.tensor_mul(oap, oap, msl)
                    else:
                        nc.vector.tensor_mul(oap, oap, msl)
                st_flag = (idx == 0)
                sp_flag = (idx == n_items - 1)
                nc.tensor.matmul(pvA[:, qlo:SQ], Vaug[:, sk, 0:128],
                                 pAB[:, qlo:SQ],
                                 start=st_flag, stop=sp_flag)
                nc.tensor.matmul(pvB[:, qlo:SQ], Vaug[:, sk, 64:192],
                                 pAB[:, SQ + qlo:2 * SQ],
                                 start=st_flag, stop=sp_flag)

            def emit_post(qb, pvA, pvB):
                qsl = slice(qb * SQ, (qb + 1) * SQ)
                if not schedule[qb]:
                    return
                nc.vector.tensor_copy(attnA[:, qsl], pvA[:, 0:SQ])
                nc.vector.tensor_copy(attnB[:, qsl], pvB[:, 0:SQ])

            def emit_oproj_st(qb, j):
                    st = 4 * qb + j
                    sl = slice(st * 128, (st + 1) * 128)
                    oA = ps2.tile([128, NF], F32, tag="ps")
                    oB = ps2.tile([128, NF], F32, tag="ps")
                    # denominators: 1-contraction matmuls pull the den
                    # rows (attnA row 64 / attnB row 0) into
                    # per-partition layout, borrowing col 0 of the
                    # oA/oB banks before the projection clobbers them
                    # (the recip read -> matmul WAR dep serializes).
                    rA = xs.tile([128, 1], F32, tag="r", bufs=4)
                    rB = xs.tile([128, 1], F32, tag="r", bufs=4)
                    nc.tensor.matmul(oA[:, 0:1], attnA[64:65, sl],
                                     ones_sb[64:65, :],
                                     start=True, stop=True)
                    nc.vector.reciprocal(rA, oA[:, 0:1])
                    nc.tensor.matmul(oB[:, 0:1], attnB[0:1, sl],
                                     ones_sb[0:1, :],
                                     start=True, stop=True)
                    nc.vector.reciprocal(rB, oB[:, 0:1])
                    nc.tensor.matmul(oA, attnA[0:64, sl], wo_sb[0:64, :],
                                     start=True, stop=True)
                    nc.tensor.matmul(oB, attnB[64:128, sl],
                                     wo_sb[64:128, :],
                                     start=True, stop=True)
                    t1 = xs.tile([128, NF], F32, tag="t1", bufs=2)
                    nc.vector.tensor_scalar_mul(t1, oB, rB)
                    osb = xs.tile([128, NF], F32, tag="os", bufs=2)
                    nc.vector.scalar_tensor_tensor(
                        osb, in0=oA, scalar=rA, in1=t1,
                        op0=ALU.mult, op1=ALU.add)
                    nc.sync.dma_start(o_d[sl, :], osb)

            # Per-rep emission, v6 ordering (best measured on HW): project
            # the next block ahead of the current block's attention, run
            # the output projection one block behind as a tail burst (its
            # den->recip->matmul chain then overlaps the next block's
            # independent attention work instead of head-of-line-blocking
            # the PE FIFO mid-stream).
            for _rep in range(reps):
                t0 = proj_dma(0)
                proj_mm(0, t0)
                for qb in range(N_QB):
                    pvA = pvp.tile([128, SQ], F32, tag="pv")
                    pvB = pvp.tile([128, SQ], F32, tag="pv")
                    if qb + 1 < N_QB:
                        t = proj_dma(qb + 1)
                        proj_mm(qb + 1, t)
                    n_items = len(schedule[qb])
                    for idx in range(n_items):
                        emit_item(qb, idx, n_items, pvA, pvB)
                    emit_post(qb, pvA, pvB)
                    if qb >= 1:
                        for j in range(4):
                            emit_oproj_st(qb - 1, j)
                for j in range(4):
                    emit_oproj_st(N_QB - 1, j)

    nc.compile()
    return nc


def _prep_core_inputs(c, q, k, v, wq, bq, wk, bk, wv, patterns):
    b = c // 4
    hp = c % 4
    cols = slice(128 * hp, 128 * (hp + 1))
    n_pat = patterns.shape[0]
    bf = ml_dtypes.bfloat16
    wo_slice = _prep_core_inputs._wo[cols, :]  # [128, 512]
    # patterns [n_pat, SK, SQ] -> [SK, n_pat, 2, SQ] (duplicated per head)
    mskd = np.repeat(patterns.transpose(1, 0, 2)[:, :, None, :], 2, axis=2)
    return {
        "qT": np.ascontiguousarray(q[b].T).astype(bf),
        "kT": np.ascontiguousarray(k[b].T).astype(bf),
        "vT": np.ascontiguousarray(v[b].T).astype(bf),
        "wq": np.ascontiguousarray(wq[:, cols]).astype(bf),
        "wk": np.ascontiguousarray(wk[:, cols]).astype(bf),
        "wv": np.ascontiguousarray(wv[:, cols]).astype(bf),
        "wo": np.ascontiguousarray(wo_slice).astype(bf),
        "bq": np.ascontiguousarray(bq[cols].reshape(128, 1)),
        "bk": np.ascontiguousarray(bk[cols].reshape(128, 1)),
        "msk": np.ascontiguousarray(
            mskd.reshape(SK, n_pat * 2 * SQ)).astype(bf),
    }


def get_state(mask_np, reps=1):
    """Build (or fetch cached) compiled program + schedule for this mask."""
    mask2d = np.asarray(mask_np, dtype=np.float32).reshape(S, S)
    schedule, patterns = _classify_mask(mask2d)
    key = (schedule, patterns.tobytes(), reps)
    if key not in _CACHE:
        nc = _build_program(schedule, patterns.shape[0], reps=reps)
        _CACHE[key] = {"nc": nc, "schedule": schedule, "patterns": patterns}
    return _CACHE[key]


def kernel(q, k, v, mask, wq, bq, wk, bk, wv, bv, wo, bo):
    q = np.asarray(q, np.float32)
    k = np.asarray(k, np.float32)
    v = np.asarray(v, np.float32)
    wq_n = np.asarray(wq, np.float32)
    wk_n = np.asarray(wk, np.float32)
    wv_n = np.asarray(wv, np.float32)
    wo_n = np.asarray(wo, np.float32)
    bq_n = np.asarray(bq, np.float32)
    bk_n = np.asarray(bk, np.float32)
    bv_n = np.asarray(bv, np.float32)
    bo_n = np.asarray(bo, np.float32)

    state = get_state(mask)
    nc = state["nc"]
    patterns = state["patterns"]

    _prep_core_inputs._wo = wo_n
    in_maps = [
        _prep_core_inputs(c, q, k, v, wq_n, bq_n, wk_n, bk_n, wv_n, patterns)
        for c in range(N_CORES)
    ]
    results = bass2jax.run_bass_via_pjrt(nc, in_maps, n_cores=N_CORES)

    bo_eff = bv_n @ wo_n + bo_n  # exact: softmax rows sum to 1
    out = np.empty((B, S, NF), np.float32)
    for b in range(B):
        acc = results[b * 4 + 0]["o"].astype(np.float32)
        for hp in range(1, 4):
            acc = acc + results[b * 4 + hp]["o"]
        out[b] = acc + bo_eff
    return out
